# revision 14
# baseline (speedup 1.0000x reference)
"""Causal attention kernel for Trainium2, 8 NeuronCores.

Problem: x[4,2048,2048] @ Wq/Wk/Wv[2048,2048] -> causal softmax attention.

Sharding: 2 cores per batch; each core owns 1024 query rows, assigned as
global 512-row chunks {0,3} (even cores) / {1,2} (odd cores) to balance causal
work.

Algebraic restructure vs the plain q/k form: scores = (x Wq)(x Wk)^T
= x (Wq Wk^T) x^T, so the kernel precomputes M = Wq Wk^T ONCE (it is shared
by every batch and query row): each core computes a 256-row slice of M
(2.15 GF) and an 8-way AllGather assembles the full M while the V projection
runs. Each core then computes y^T = M^T x_own^T (one projection instead of
Q and K — halves that matmul cost) and scores S^T = x_keys y^T, where x_keys
is just the INPUT x^T read from HBM in pair-rank key order — no K AllGather
at all. V is projected per-core and pair-gathered as before.

Key order for phase 2 is the pair-rank permutation [chunk0, chunk3, chunk1,
chunk2] (identical on every core, SPMD-uniform): query slot 0 (chunk c_lo)
attends to key positions {0-3, 8-11}, slot 1 (chunk c_hi) to all 16; true
causality for the permuted order is enforced by per-core mask tensors.

On-device layout: scores are computed transposed (S^T, keys on partitions) so
exp(S^T) feeds the attention@V matmul directly as the stationary operand with
no transpose; row sums come from a matmul against a ones vector; softmax
max-subtraction is skipped (shift-invariant; scores are O(1) so fp32 range is
ample).

dtypes: all matmul operands bf16 (fp32 PSUM accumulation); fp8 was measured
(host sim) at 4e-2 final rel err — over the 2e-2 gate — so bf16 stays.
"""

import math

import numpy as np
import ml_dtypes

import concourse.bass as bass
import concourse.mybir as mybir
import concourse.tile as tile
from concourse import bacc
from concourse.bass import ds, ts
from concourse.bass_utils import run_bass_kernel_spmd

B, S, D = 4, 2048, 2048
P = 128
DC = D // P          # 16 contraction chunks
SB = S // P          # 16 key blocks
QROWS = 1024         # query rows per core
NCORES = 8
MROWS = 256          # rows of M computed per core
INV_SQRT_D = 1.0 / math.sqrt(D)

# gathered key-block position -> true 512-chunk (pair-rank order, all cores)
POS2TRUE = [0, 3, 1, 2]
# key-block positions processed by query slot 0
SLOT0_POS = [0, 1, 2, 3, 8, 9, 10, 11]
S0IDX = {pos: j for j, pos in enumerate(SLOT0_POS)}
PAIRS = [[0, 1], [2, 3], [4, 5], [6, 7]]
ALL8 = [list(range(NCORES))]

F32 = mybir.dt.float32
BF16 = mybir.dt.bfloat16
XW_DT = BF16    # x^T and W inputs + projection matmuls
QK_DT = BF16    # y^T / x_keys staging + score matmuls
Exp = mybir.ActivationFunctionType.Exp

_CACHED_NC = None


def build_nc():
    global _CACHED_NC
    if _CACHED_NC is not None:
        return _CACHED_NC
    nc = bacc.Bacc(trn_type="TRN2", target_bir_lowering=False, debug=False,
                   num_devices=NCORES)

    xt_d = nc.dram_tensor("xt", [D, QROWS], XW_DT, kind="ExternalInput")
    xk_d = nc.dram_tensor("xk", [P, DC, SB, P], QK_DT, kind="ExternalInput")
    wq_d = nc.dram_tensor("wq", [P, DC, MROWS], XW_DT, kind="ExternalInput")
    wk_d = nc.dram_tensor("wk", [P, DC, S], XW_DT, kind="ExternalInput")
    wv_d = nc.dram_tensor("wv", [4, 2, P, 8, 512], XW_DT, kind="ExternalInput")
    mk_d = nc.dram_tensor("masks", [P, 24, 512], BF16, kind="ExternalInput")
    out_d = nc.dram_tensor("out", [QROWS, D], F32, kind="ExternalOutput")

    with tile.TileContext(nc) as tc:
        with (
            tc.tile_pool(name="dram", bufs=1, space="DRAM") as dpool,
            tc.tile_pool(name="ps", bufs=8, space="PSUM") as ps_all,
        ):
            qT = dpool.tile([P, DC, QROWS], QK_DT, tag="qT")      # [p, d2, q]
            # M slices: own rows, split in two d2-halves so each AllGather
            # can complete (and unblock y^T chunks) independently
            m_own = [dpool.tile([2, P, 2, 512], QK_DT, tag=f"mown{g}",
                                name=f"mown{g}") for g in range(2)]
            m_all = [dpool.tile([NCORES, 2, P, 2, 512], QK_DT, tag=f"mall{g}",
                                name=f"mall{g}", addr_space="Shared")
                     for g in range(2)]
            vv_own = [dpool.tile([2, P, D], BF16, tag=f"vvo{g}",
                                 name=f"vvo{g}") for g in range(4)]
            vgs = [dpool.tile([2, 2, P, D], BF16, tag=f"vg{g}",
                              name=f"vg{g}") for g in range(4)]

            # ---------------- phase 1: M slice + V + y^T ----------------
            with (
                tc.tile_pool(name="xt", bufs=1) as xt_pool,
                tc.tile_pool(name="wk", bufs=1) as wk_pool,
                tc.tile_pool(name="wq", bufs=1) as wq_pool,
                tc.tile_pool(name="st", bufs=16) as st_pool,
            ):
                # Wk^T (full) + Wq^T (own 256 M-rows) for the M slice,
                # interleaved per-dc so the dc-outer M matmuls can follow
                # right behind the DMA front.
                wkt = wk_pool.tile([P, DC, S], XW_DT, tag="wkt")
                wqo = wq_pool.tile([P, DC, MROWS], XW_DT, tag="wqo")
                for dc in range(DC):
                    nc.sync.dma_start(wqo[:, dc, :], wq_d.ap()[:, dc, :])
                    nc.sync.dma_start(wkt[:, dc, :], wk_d.ap()[:, dc, :])

                xts = [xt_pool.tile([P, DC, 512], XW_DT, tag=f"xt{c}",
                                    name=f"xt{c}")
                       for c in range(2)]
                for c in range(2):
                    for dc in range(DC):
                        nc.sync.dma_start(xts[c][:, dc, :],
                                          xt_d.ap()[ds(dc * P, P), ts(c, 512)])

                # --- M slice: M[own 256 rows, :] = Wq_own @ Wk^T, in two
                # d2-half batches of 4 concurrent PSUM groups (dc-outer
                # inside a batch), INTERLEAVED with quarter-blocks of the V
                # projection. Each M half is followed by its copies + its
                # AllGather, so the first gather fires ~15µs in and both are
                # done long before y^T; the V n-blocks in between keep the
                # wv DMA demand smooth (2MB per block, not 8MB up front).
                with (
                    tc.tile_pool(name="wv", bufs=8) as wv_pool,
                    tc.tile_pool(name="stv", bufs=8) as stv_pool,
                ):
                    def load_wv(n):
                        tiles = []
                        for hb in range(2):
                            wvt = wv_pool.tile([P, 8, 512], XW_DT, tag="wv",
                                               name=f"wv{n}{hb}")
                            nc.sync.dma_start(wvt[:, :4, :],
                                              wv_d.ap()[n, hb][:, :4, :])
                            nc.sync.dma_start(wvt[:, 4:, :],
                                              wv_d.ap()[n, hb][:, 4:, :])
                            tiles.append(wvt)
                        return tiles

                    def m_half(g):
                        mps = [ps_all.tile([P, 512], F32, tag="ps",
                                           name=f"mps{g}{i}") for i in range(4)]
                        for dc in range(DC):
                            for rc in range(2):
                                for cs in range(2):
                                    nc.tensor.matmul(
                                        mps[rc * 2 + cs][:],
                                        lhsT=wqo[:, dc, ts(rc, P)],
                                        rhs=wkt[:, dc, ts(2 * g + cs, 512)],
                                        start=(dc == 0), stop=(dc == DC - 1),
                                    )
                        for rc in range(2):
                            for cs in range(2):
                                st = st_pool.tile([P, 512], QK_DT, tag="st")
                                nc.scalar.copy(st[:], mps[rc * 2 + cs][:])
                                nc.gpsimd.dma_start(m_own[g][rc, :, cs, :],
                                                    st[:])
                        nc.gpsimd.collective_compute(
                            "AllGather", mybir.AluOpType.bypass,
                            replica_groups=ALL8,
                            ins=[m_own[g].opt()],
                            outs=[m_all[g].opt()],
                        )

                    def v_block(n, wva, wvb):
                        for s in range(8):
                            ps = ps_all.tile([P, 512], F32, tag="ps")
                            for dc in range(DC):
                                w = wva if dc < 8 else wvb
                                nc.tensor.matmul(
                                    ps[:], lhsT=xts[s // 4][:, dc, ts(s % 4, P)],
                                    rhs=w[:, dc % 8, :],
                                    start=(dc == 0), stop=(dc == DC - 1),
                                )
                            sv = stv_pool.tile([P, 512], BF16, tag="sv")
                            nc.vector.tensor_copy(sv[:], ps[:])
                            nc.gpsimd.dma_start(
                                vv_own[s // 2][s % 2, :, ts(n, 512)], sv[:])

                    wv0 = load_wv(0)
                    m_half(0)
                    wv1 = load_wv(1)
                    v_block(0, *wv0)
                    m_half(1)
                    wv2 = load_wv(2)
                    v_block(1, *wv1)
                    wv3 = load_wv(3)
                    v_block(2, *wv2)
                    v_block(3, *wv3)
                    for g in range(4):
                        nc.gpsimd.collective_compute(
                            "AllGather", mybir.AluOpType.bypass,
                            replica_groups=PAIRS,
                            ins=[vv_own[g].opt()],
                            outs=[vgs[g].opt()],
                        )

                # --- y^T = M^T x_own^T (needs the gathered M)
                with tc.tile_pool(name="mw", bufs=4) as mw_pool:
                    for mb in range(DC):
                        mw = mw_pool.tile([P, DC, P], QK_DT, tag="mw",
                                          name=f"mw{mb}")
                        mg = m_all[0] if mb < 8 else m_all[1]
                        for dc1 in range(DC):
                            nc.sync.dma_start(
                                mw[:, dc1, :],
                                mg[dc1 // 2, dc1 % 2, :, (mb // 4) % 2,
                                   ts(mb % 4, P)])
                        for s in range(2):
                            ps = ps_all.tile([P, 512], F32, tag="ps")
                            for dc1 in range(DC):
                                nc.tensor.matmul(
                                    ps[:], lhsT=mw[:, dc1, :],
                                    rhs=xts[s][:, dc1, :],
                                    start=(dc1 == 0), stop=(dc1 == DC - 1),
                                )
                            st = st_pool.tile([P, 512], QK_DT, tag="st")
                            nc.scalar.copy(st[:], ps[:])
                            nc.gpsimd.dma_start(qT[:, mb, ts(s, 512)], st[:])

            # ---------------- phase 2: attention ----------------
            with (
                tc.tile_pool(name="pt", bufs=1) as pt_pool,
                tc.tile_pool(name="mk", bufs=1) as mk_pool,
                tc.tile_pool(name="vb", bufs=2) as vb_pool,
                tc.tile_pool(name="kt2", bufs=13) as kt_pool,
                tc.tile_pool(name="qtv", bufs=2) as qtv_pool,
                tc.tile_pool(name="one", bufs=1) as one_pool,
                tc.tile_pool(name="sc", bufs=4) as sc_pool,
                tc.tile_pool(name="ob", bufs=4) as ob_pool,
            ):
                mk = mk_pool.tile([P, 24, 512], BF16, tag="mk")
                for j in range(3):
                    nc.sync.dma_start(mk[:, ts(j, 8), :], mk_d.ap()[:, ts(j, 8), :])
                ones = one_pool.tile([P, 1], BF16, tag="ones")
                nc.vector.memset(ones[:], 1.0)
                # pt index: slot0 j -> key pos SLOT0_POS[j]; slot1 kb -> 8+kb
                pt = pt_pool.tile([P, 24, 512], BF16, tag="pt")

                # --- scores + exp.  qt/vt share pool slots (tag "qt"): the
                # two V super-tiles allocate into the slots the q tiles
                # release after their last score matmul.
                qts = []
                for slot in range(2):
                    qt = qtv_pool.tile([P, DC, 512], QK_DT, tag="qt",
                                       name=f"qt{slot}")
                    for j in range(4):
                        nc.sync.dma_start(qt[:, ts(j, 4), :],
                                          qT[:, ts(j, 4), ts(slot, 512)])
                    qts.append(qt)

                # V super-tiles for AV: triggered from the vector queue so
                # the 16MB streams in while scores run (the sync queue's kt
                # reloads are score-gated and would delay them past scores)
                vbig = [vb_pool.tile([P, 8, D], BF16, tag="vb",
                                     name=f"vb{half}") for half in range(2)]
                vb_order = ([(0, j) for j in range(4)] + [(1, 0)]
                            + [(0, j) for j in range(4, 8)]
                            + [(1, j) for j in range(1, 8)])
                for half, j in vb_order:
                    nc.scalar.dma_start(vbig[half][:, j, :],
                                        vgs[j // 2][half, j % 2])

                for kb in range(SB):
                    kt_t = kt_pool.tile([P, DC, P], QK_DT, tag="kt",
                                        name=f"kt{kb}")
                    for j in range(4):
                        nc.sync.dma_start(kt_t[:, ts(j, 4), :],
                                          xk_d.ap()[:, ts(j, 4), kb, :])
                    targets = []
                    if kb in S0IDX:
                        targets.append((S0IDX[kb], 0))
                    targets.append((8 + kb, 1))
                    for pti, slot in targets:
                        ps = ps_all.tile([P, 512], F32, tag="ps")
                        for dc in range(DC):
                            nc.tensor.matmul(
                                ps[:], lhsT=kt_t[:, dc, :],
                                rhs=qts[slot][:, dc, :],
                                start=(dc == 0), stop=(dc == DC - 1),
                            )
                        nc.scalar.activation(pt[:, pti, :], ps[:], Exp,
                                             scale=INV_SQRT_D)

                def vt_ap(pos):
                    return vbig[pos // 8][:, pos % 8, :]

                for j in range(24):
                    nc.vector.tensor_mul(pt[:, j, :], pt[:, j, :], mk[:, j, :])

                for slot in range(2):
                    for qs in range(4):
                        # per-qs causal union over the pair: key blocks that
                        # are fully masked on BOTH cores are skipped
                        if slot == 0:
                            kpos = [0, 1, 2, 3] + list(range(8, 9 + qs))
                            idx = [S0IDX[p] for p in kpos]
                        else:
                            kpos = ([0, 1, 2, 3] + list(range(4, 5 + qs))
                                    + list(range(8, 16)))
                            idx = [8 + p for p in kpos]
                        # row-sum matmuls (N=1) interleave with the 4 AV
                        # matmuls sharing the same stationary operand, so
                        # their LDWEIGHTS hides under the N=512 streams
                        plt = ps_all.tile([P, 512], F32, tag="ps", name="pl")
                        pl = plt[:, :1]
                        pavs = [ps_all.tile([P, 512], F32, tag="ps",
                                            name=f"pav{n}") for n in range(4)]
                        for i, j in enumerate(idx):
                            se = dict(start=(i == 0), stop=(i == len(idx) - 1))
                            nc.tensor.matmul(pl[:], lhsT=pt[:, j, ts(qs, P)],
                                             rhs=ones[:], **se)
                            for n in range(4):
                                nc.tensor.matmul(
                                    pavs[n][:], lhsT=pt[:, j, ts(qs, P)],
                                    rhs=vt_ap(kpos[i])[:, ts(n, 512)], **se)
                        rl = sc_pool.tile([P, 1], F32, tag="rl")
                        nc.vector.reciprocal(rl[:], pl[:])
                        for n in range(4):
                            pav = pavs[n]
                            ob = ob_pool.tile([P, 512], F32, tag="ob")
                            if n % 2 == 0:
                                nc.vector.tensor_scalar_mul(ob[:], pav[:],
                                                            rl[:])
                            else:
                                nc.scalar.activation(
                                    ob[:], pav[:],
                                    mybir.ActivationFunctionType.Copy,
                                    scale=rl[:])
                            nc.sync.dma_start(
                                out_d.ap()[ds(slot * 512 + qs * P, P),
                                           ts(n, 512)],
                                ob[:],
                            )

    nc.compile()
    _CACHED_NC = nc
    return nc


def _host_prep(x, Wq, Wk, Wv):
    """Build per-core input maps (host-side layout prep)."""
    np_xw = ml_dtypes.bfloat16 if XW_DT == BF16 else np.float32
    # Wk^T in [P, DC, S] chunk layout (same for every core)
    wk_h = np.ascontiguousarray(
        Wk.T.reshape(DC, P, S).transpose(1, 0, 2)).astype(np_xw)
    wv_h = np.ascontiguousarray(
        Wv.reshape(2, 8, P, 4, 512).transpose(3, 0, 2, 1, 4)).astype(np_xw)
    # per-core 256-row slice of Wq, transposed to [P, DC, 256]
    wq_h = [np.ascontiguousarray(
        Wq[r * MROWS:(r + 1) * MROWS, :].T.reshape(DC, P, MROWS)
        .transpose(1, 0, 2)).astype(np_xw) for r in range(NCORES)]

    k_in_block = np.arange(P, dtype=np.int64)[:, None]           # [P, 1]
    q_in_chunk = np.arange(512, dtype=np.int64)[None, :]         # [1, 512]

    def build_masks(h):
        c_lo, c_hi = h, 3 - h
        masks = np.zeros((P, 24, 512), dtype=ml_dtypes.bfloat16)
        for j, pos in enumerate(SLOT0_POS):
            tkb = POS2TRUE[pos // 4] * 4 + pos % 4
            masks[:, j, :] = (tkb * P + k_in_block) <= (c_lo * 512 + q_in_chunk)
        for pos in range(SB):
            tkb = POS2TRUE[pos // 4] * 4 + pos % 4
            masks[:, 8 + pos, :] = (tkb * P + k_in_block) <= (c_hi * 512 + q_in_chunk)
        return masks

    mask_h = [build_masks(0), build_masks(1)]

    # per-batch x^T and its pair-rank-permuted key layout [P, DC, SB, P]
    xt_b, xk_b = [], []
    for b in range(B):
        xt = x[b].T                                              # [D, S] view
        xkc = np.concatenate(
            [xt[:, 0:512], xt[:, 1536:2048],
             xt[:, 512:1024], xt[:, 1024:1536]], axis=1)
        xk_b.append(np.ascontiguousarray(
            xkc.reshape(DC, P, SB, P).transpose(1, 0, 2, 3)).astype(np_xw))
        xt_b.append(xt)

    in_maps = []
    for core in range(NCORES):
        b, h = divmod(core, 2)
        c_lo, c_hi = h, 3 - h
        xt = xt_b[b]
        xtp = np.concatenate(
            [xt[:, c_lo * 512:(c_lo + 1) * 512],
             xt[:, c_hi * 512:(c_hi + 1) * 512]], axis=1)         # [D, 1024]
        in_maps.append({
            "xt": np.ascontiguousarray(xtp).astype(np_xw),
            "xk": xk_b[b],
            "wq": wq_h[core], "wk": wk_h, "wv": wv_h, "masks": mask_h[h],
        })
    return in_maps


def run(x, Wq, Wk, Wv, trace=False):
    x = np.asarray(x, dtype=np.float32)
    Wq = np.asarray(Wq, dtype=np.float32)
    Wk = np.asarray(Wk, dtype=np.float32)
    Wv = np.asarray(Wv, dtype=np.float32)
    nc = build_nc()
    in_maps = _host_prep(x, Wq, Wk, Wv)
    res = run_bass_kernel_spmd(nc, in_maps, core_ids=list(range(NCORES)),
                               trace=trace)
    out = np.empty((B, S, D), dtype=np.float32)
    for core in range(NCORES):
        b, h = divmod(core, 2)
        c_lo, c_hi = h, 3 - h
        o = res.results[core]["out"]
        out[b, c_lo * 512:(c_lo + 1) * 512] = o[:512]
        out[b, c_hi * 512:(c_hi + 1) * 512] = o[512:]
    return out, res


def kernel(x, Wq, Wk, Wv):
    out, _ = run(x, Wq, Wk, Wv)
    return out


if __name__ == "__main__":
    build_nc()
    print("build + compile OK")
